# revision 69
# baseline (speedup 1.0000x reference)
"""Causal self-attention on 8 trn2 NeuronCores.

Sharding: tensor-parallel over heads. Core c computes Q/K/V and attention
for heads {2c, 2c+1} over all batches (column-parallel W_q/W_k/W_v slices),
then per-batch AllToAlls redistribute the attention outputs token-wise so
each core runs the full output projection (contraction over all 16 heads'
features) for its 1/8 slice of each batch's tokens.

Core layout choices (per core):
 - QKV projections run in fp8 DoubleRow mode (K_eff=256/pass): x is split
   host-side into hi+lo e4m3 parts (hi+lo ~ bf16 accuracy), weights are
   single e4m3; 8 DoubleRow passes replace 8 bf16 passes at half cost.
 - Q/K are stored transposed in fp8 with a zeroed second DoubleRow slot:
   score matmuls [k,q] run DoubleRow over (real, zero) slots, halving the
   per-column cost despite the hd=64 contraction.
 - Scores psum is [128k, 2 heads, 512q]; one Exp activation covers both
   heads (bias = key-padding per partition, scale = 1/sqrt(hd)).
 - P@V runs q-major: probs block [128k,128q] is the stationary operand,
   V [128k, 64f|1] the moving one, accumulating [128q, 4sb, 65] in psum.
   The ones-column yields per-q (per-partition) softmax denominators, so
   normalization is one reciprocal + one broadcast-multiply on DVE.
 - att is stored token-major; per-batch AllToAlls move 128-token blocks,
   and the receiver transposes [tok,f]->[f,tok] with XBAR DMA transposes
   feeding the bf16 output projection. Biases ride DVE copies (no bias
   matmuls anywhere).
 - Causal masking is structural: fully-masked regions are never computed
   (restricted matmul/exp column ranges); the diagonal 128x128 triangle
   is a 0/1 multiply on GpSimd.
"""

import numpy as np
import ml_dtypes

import concourse.bass as bass
import concourse.mybir as mybir
import concourse.tile as tile
from concourse import bacc
from concourse.bass_utils import run_bass_kernel_spmd

B, L, D, H, HD = 4, 2048, 1024, 16, 64
NCORES = 8
DL = 128              # local feature dim: 2 heads * 64
BL = B * L            # 8192
SCALE = HD ** -0.5
NEG = -1e9

QT = 512              # query tile (free dim)
KB = 128              # key block (partition dim)
NQT = L // QT         # 4 q-tiles per batch
NKB = L // KB         # 16 k-blocks per batch
NTB = BL // KB        # 64 token blocks of 128

FP32 = mybir.dt.float32
BF16 = mybir.dt.bfloat16
FP8 = mybir.dt.float8e4
EXP = mybir.ActivationFunctionType.Exp
DR = mybir.MatmulPerfMode.DoubleRow

TRACE = False
LAST_EXEC_NS = None
_CACHED_NC = None
_SIM_MODE = False   # replace collectives with local DMAs; 1 device

# A2A is per q-tile: chunk ck = 4*b + qt covers that q-tile's 512 tokens;
# core c receives tokens [2048b + 512qt + 64c : +64) (64-row A2A chunks)
# and computes their full output projection, transposed ([dout, tok]).


def build_program():
    nc = bacc.Bacc("TRN2", target_bir_lowering=False, debug=False,
                   num_devices=(1 if _SIM_MODE else NCORES))
    xT_hl = nc.dram_tensor("xT_hl", [2 * D, BL], FP8, kind="ExternalInput").ap()
    wq8 = nc.dram_tensor("wq8", [D, DL], FP8, kind="ExternalInput").ap()
    wk8 = nc.dram_tensor("wk8", [D, DL], FP8, kind="ExternalInput").ap()
    wv8 = nc.dram_tensor("wv8", [D, DL], FP8, kind="ExternalInput").ap()
    wv8_lo = nc.dram_tensor("wv8_lo", [D, DL], FP8, kind="ExternalInput").ap()
    wo_t = nc.dram_tensor("wo_t", [D, D], BF16, kind="ExternalInput").ap()
    bq_c = nc.dram_tensor("bq_c", [DL, 1], FP32, kind="ExternalInput").ap()
    bk_c = nc.dram_tensor("bk_c", [DL, 1], FP32, kind="ExternalInput").ap()
    bv4_r = nc.dram_tensor("bv4_r", [1, 4 * DL], FP32, kind="ExternalInput").ap()
    bo_r = nc.dram_tensor("bo_r", [1, D], FP32, kind="ExternalInput").ap()
    pad_b = nc.dram_tensor("pad_b", [KB, B * NKB], FP32, kind="ExternalInput").ap()
    cmask2 = nc.dram_tensor("cmask2", [KB, 2 * KB], BF16, kind="ExternalInput").ap()
    out_chunkT = nc.dram_tensor("out_chunkT", [D, BL // NCORES], BF16,
                                kind="ExternalOutput").ap()

    with tile.TileContext(nc) as tc:
        with tc.tile_pool(name="persist", bufs=1) as persist, \
             tc.tile_pool(name="xpool", bufs=8) as xpool, \
             tc.tile_pool(name="probs", bufs=5) as probs, \
             tc.tile_pool(name="small", bufs=4) as small, \
             tc.tile_pool(name="opool", bufs=3) as opool, \
             tc.tile_pool(name="gpool", bufs=3) as gpool, \
             tc.tile_pool(name="ps", bufs=1, space="PSUM") as ps, \
             tc.tile_pool(name="dram", bufs=1, space="DRAM") as dram:

            # ---- persistent tiles ----
            wq_sb = persist.tile([128, 4, 2, 128], FP8)
            wk_sb = persist.tile([128, 4, 2, 128], FP8)
            wv_sb = persist.tile([128, 4, 2, 128], FP8)
            wvlo_sb = persist.tile([128, 4, 2, 128], FP8)
            wo_sb = persist.tile([128, 8, D], BF16)
            cm2_sb = persist.tile([KB, 2, KB], BF16)
            pad_sb = persist.tile([KB, B * NKB], FP32)
            bq_sb = persist.tile([DL, 1], FP32)
            bk_sb = persist.tile([DL, 1], FP32)
            bv_sb = persist.tile([128, 4, 2, 64], FP32)
            bo_sb = persist.tile([128, 8, 1], FP32)    # per-dout-partition
            QT_sb = persist.tile([128, 2, BL], FP8)    # [2h x 64hd, slot, tok]
            KT_sb = persist.tile([128, 2, BL], FP8)
            V_sb = persist.tile([128, NTB, 130], BF16)  # [key, kblk, VA|1|VB|1]
            att_sb = persist.tile([128, NTB, DL], BF16)  # token-major

            # zero DoubleRow slots / V ones-columns. Split across engines
            # and batches so the first scores matmul isn't gated ~20us:
            # batch 0's slice inline, later batches as background items.
            nc.vector.memset(QT_sb[:, 1, 0:L], 0.0)
            nc.gpsimd.memset(KT_sb[:, 1, 0:L], 0.0)
            nc.gpsimd.memset(V_sb[:, :, 64:65], 1.0)    # ones cols only —
            nc.gpsimd.memset(V_sb[:, :, 129:130], 1.0)  # rest is overwritten
            # trigger the Exp act-table load off the critical path
            warm = small.tile([1, 1], FP32, tag="warm")
            nc.vector.memset(warm, 0.0)
            warm2 = small.tile([1, 1], BF16, tag="warm2")
            nc.scalar.activation(warm2, warm, EXP)

            # ---- background PE work: QKV chunks / out-proj tiles ----
            def load_weights_qkv():
                nc.sync.dma_start(out=wq_sb, in_=wq8.rearrange(
                    "(t s p) m -> p t s m", p=128, s=2))
                nc.sync.dma_start(out=wk_sb, in_=wk8.rearrange(
                    "(t s p) m -> p t s m", p=128, s=2))
                nc.sync.dma_start(out=wv_sb, in_=wv8.rearrange(
                    "(t s p) m -> p t s m", p=128, s=2))
                nc.sync.dma_start(out=wvlo_sb, in_=wv8_lo.rearrange(
                    "(t s p) m -> p t s m", p=128, s=2))

            def load_consts():
                # small consts ride the (otherwise idle) Act HWDGE queue so
                # they don't serialize behind the big x/weight loads on SP
                nc.scalar.dma_start(out=bq_sb, in_=bq_c)
                nc.scalar.dma_start(out=bk_sb, in_=bk_c)
                nc.scalar.dma_start(out=pad_sb, in_=pad_b)
                nc.scalar.dma_start(out=cm2_sb, in_=cmask2.rearrange(
                    "p (h q) -> p h q", h=2))
                nc.scalar.dma_start(out=bv_sb, in_=bv4_r.rearrange(
                    "o (v h f) -> o v h f", v=4, h=2).to_broadcast(
                        [128, 4, 2, 64]))

            def load_wo():
                nc.sync.dma_start(out=wo_sb, in_=wo_t.rearrange(
                    "(t p) m -> p t m", p=128))

            def load_bo():
                nc.sync.dma_start(out=bo_sb, in_=bo_r.rearrange(
                    "o (db p) -> p db o", p=128))

            def emit_xt_dma(lc, split=False):
                xt = xpool.tile([128, 2, 4, 2, QT], FP8, tag="xt",
                                name=f"xt{lc}")
                cols = slice(QT * lc, QT * (lc + 1))
                src = xT_hl[:, cols].rearrange(
                    "(u t s p) l -> p u t s l", p=128, s=2, t=4)
                if split:   # hi first so the first matmuls start sooner
                    nc.sync.dma_start(out=xt[:, 0], in_=src[:, 0])
                    nc.sync.dma_start(out=xt[:, 1], in_=src[:, 1])
                else:
                    nc.sync.dma_start(out=xt, in_=src)
                return xt[:, 0], xt[:, 1]

            xts = {}

            def emit_chunk_qk(lc, which):
                xh, xl = xts[lc]
                w_sb, b_sb, dst = ((wq_sb, bq_sb, QT_sb) if which == "q"
                                   else (wk_sb, bk_sb, KT_sb))
                p = ps.tile([128, QT], FP32, tag="mm", bufs=2, name=f"p{which}{lc}")
                for pi, xt in enumerate((xh, xl)):
                    for t in range(4):
                        nc.tensor.matmul(p, lhsT=w_sb[:, t], rhs=xt[:, t],
                                         perf_mode=DR,
                                         start=(pi == 0 and t == 0),
                                         stop=(pi == 1 and t == 3))
                nc.vector.tensor_scalar_add(
                    dst[:, 0, QT * lc:QT * (lc + 1)], p, b_sb)

            def emit_chunk_v(lc):
                # psum accumulates 16*V: (xh+xl) @ (16*wv_hi) + xh @ wv_lo16
                # (the W residual is pre-scaled x16 host-side so it lands in
                # e4m3's range); one DVE op then does psum/16 + bias.
                xh, xl = xts[lc]
                p = ps.tile([128, 4, 128], FP32, tag="mm", bufs=2, name=f"pv{lc}")
                # one accumulation group for the whole 2KB psum zero-region:
                # start exactly once, stop exactly once
                for pi, xt, w_sb in ((0, xh, wv_sb), (1, xl, wv_sb),
                                     (2, xh, wvlo_sb)):
                    for t in range(4):
                        for vs in range(4):
                            nc.tensor.matmul(
                                p[:, vs, :],
                                lhsT=xt[:, t, :, KB * vs:KB * (vs + 1)],
                                rhs=w_sb[:, t], perf_mode=DR,
                                start=(pi == 0 and t == 0 and vs == 0),
                                stop=(pi == 2 and t == 3 and vs == 3))
                kt0 = 4 * lc
                nc.vector.scalar_tensor_tensor(
                    V_sb[:, kt0:kt0 + 4, :].rearrange(
                        "p v (h fo) -> p v h fo", h=2)[:, :, :, 0:64],
                    p.rearrange("p v (h f) -> p v h f", h=2),
                    1.0 / 16.0, bv_sb,
                    mybir.AluOpType.mult, mybir.AluOpType.add)

            gathT = {}

            def emit_op(ck):
                """Out-proj for chunk ck (64 tokens): out[dout, tok] =
                sum_f w_o[f, dout] att[f, tok] — wo stationary, 64-token
                gathT moving, transposed output."""
                g = gathT[ck]    # [128 f, 8 src, 64 tok]
                po = ps.tile([128, 8, 64], FP32, tag="mm", bufs=2,
                             name=f"po{ck}")
                for db in range(8):
                    for s in range(8):
                        nc.tensor.matmul(
                            po[:, db, :],
                            lhsT=wo_sb[:, s, KB * db:KB * (db + 1)],
                            rhs=g[:, s, :],
                            start=(db == 0 and s == 0),
                            stop=(db == 7 and s == 7))
                ot = opool.tile([128, 8, 64], BF16, tag="ot",
                                name=f"ot{ck}")
                nc.vector.tensor_add(ot, po,
                                     bo_sb.to_broadcast([128, 8, 64]))
                nc.sync.dma_start(
                    out=out_chunkT[:, 64 * ck:64 * (ck + 1)].rearrange(
                        "(db p) t -> p db t", p=128),
                    in_=ot)

            bg = []
            bg_slow = []         # (ready_at_pop, fn): out-proj work
            pop_count = [0]

            def mk(fn, *a):
                return lambda: fn(*a)

            def pop_bg(n=1):
                for _ in range(n):
                    pop_count[0] += 1
                    if bg:
                        bg.pop(0)()
                    elif bg_slow and bg_slow[0][0] <= pop_count[0]:
                        bg_slow.pop(0)[1]()

            # seed: QKV for batch 0 (chunks 0-3), prefetching 2 ahead.
            # xt(0) first so the first matmul isn't behind all const loads.
            xts[0] = emit_xt_dma(0, split=True)
            load_weights_qkv()
            xts[1] = emit_xt_dma(1)
            load_consts()
            for lc in range(4):
                if lc + 2 < 4:
                    xts[lc + 2] = emit_xt_dma(lc + 2)
                emit_chunk_qk(lc, "q")
                emit_chunk_qk(lc, "k")
                emit_chunk_v(lc)
            bg.append(load_wo)
            bg.append(load_bo)

            def push_batch_chunks(b):
                def dma_item(l):
                    return mk(lambda l2: xts.__setitem__(l2, emit_xt_dma(l2)), l)
                l0 = 4 * b
                cols = slice(L * b, L * (b + 1))
                bg.append(mk(nc.gpsimd.memset, QT_sb[:, 1, cols], 0.0))
                bg.append(mk(nc.gpsimd.memset, KT_sb[:, 1, cols], 0.0))
                for lc in range(l0, l0 + 4):
                    bg.append(dma_item(lc))
                for lc in range(l0, l0 + 4):
                    bg.append(mk(emit_chunk_qk, lc, "q"))
                    bg.append(mk(emit_chunk_qk, lc, "k"))
                    bg.append(mk(emit_chunk_v, lc))

            def emit_a2a(ck, tb0):
                """A2A + transpose for q-tile chunk ck (tbs tb0..tb0+4):
                64-row chunks; core c receives tokens [64c:64c+64) of the
                tile from every source core."""
                a_in = dram.tile([NCORES * 64, KB], BF16, tag=f"ai{ck}",
                                 name=f"a2a_in{ck}")
                a_out = dram.tile([NCORES * 64, KB], BF16, tag=f"ao{ck}",
                                  name=f"a2a_out{ck}")
                nc.sync.dma_start(
                    out=a_in.rearrange("(tb p) f -> p tb f", p=128),
                    in_=att_sb[:, tb0:tb0 + 4, :])
                if _SIM_MODE:
                    nc.sync.dma_start(out=a_out, in_=a_in)
                else:
                    nc.gpsimd.collective_compute(
                        "AllToAll", mybir.AluOpType.bypass,
                        replica_groups=[list(range(NCORES))],
                        ins=[a_in.opt()], outs=[a_out.opt()])
                # XBAR transpose [512 (s,tok), 128 f] -> [128 f, 8 s, 64 tok]
                # (HW semantics: out[p, e, l] = in[e*L + l, p], L=64; note
                # the interpreter disagrees — hardware-verified via probe)
                g = gpool.tile([128, 8, 64], BF16, tag="gt",
                               name=f"gathT{ck}")
                gathT[ck] = g
                nc.sync.dma_start_transpose(out=g, in_=a_out)
                bg_slow.append((pop_count[0] + 24, mk(emit_op, ck)))

            # ---- attention ----
            for b in range(B):
                if b + 1 < B:
                    push_batch_chunks(b + 1)
                for qt in range(NQT):
                    q0 = L * b + QT * qt
                    tb0 = 16 * b + 4 * qt
                    nkb = 4 * (qt + 1)
                    pva = ps.tile([128, 4, 65], FP32, tag="pva", name=f"pva{b}{qt}")
                    pvb = ps.tile([128, 4, 65], FP32, tag="pvb", name=f"pvb{b}{qt}")
                    pvs = (pva, pvb)
                    for j in range(nkb):
                        k0 = L * b + KB * j
                        kt = NKB * b + j
                        o = j - 4 * qt
                        c0 = KB * o if o >= 0 else 0
                        sc = ps.tile([128, 2, QT], FP32, tag="sc", bufs=2,
                                     name=f"sc{b}{qt}{j}")
                        for h, p0 in ((0, 0), (1, 64)):
                            nc.tensor.matmul(
                                sc[:, h, c0:], perf_mode=DR,
                                lhsT=KT_sb[p0:p0 + 64, :, k0:k0 + KB],
                                rhs=QT_sb[p0:p0 + 64, :, q0 + c0:q0 + QT],
                                start=True, stop=True)
                        pr = probs.tile([128, 2, QT], BF16, tag="pr",
                                        name=f"pr{b}{qt}{j}")
                        nc.scalar.activation(pr[:, :, c0:], sc[:, :, c0:],
                                             EXP, bias=pad_sb[:, kt:kt + 1],
                                             scale=SCALE)
                        if o >= 0:
                            nc.gpsimd.tensor_mul(pr[:, :, c0:c0 + KB],
                                                 pr[:, :, c0:c0 + KB], cm2_sb)
                        for h in range(2):
                            for sb in range(max(o, 0), 4):
                                nc.tensor.matmul(
                                    pvs[h][:, sb, :],
                                    lhsT=pr[:, h, KB * sb:KB * (sb + 1)],
                                    rhs=V_sb[:, kt, 65 * h:65 * (h + 1)],
                                    start=(j == 0 and sb == 0),
                                    stop=(j == nkb - 1 and sb == 3))
                        pop_bg()
                    for h in range(2):
                        rec = small.tile([128, 4, 1], FP32, tag="rec",
                                         name=f"rec{b}{qt}{h}")
                        nc.vector.reciprocal(rec, pvs[h][:, :, 64:65])
                        nc.vector.tensor_mul(
                            att_sb[:, tb0:tb0 + 4, 64 * h:64 * (h + 1)],
                            pvs[h][:, :, 0:64],
                            rec.to_broadcast([128, 4, 64]))
                    emit_a2a(4 * b + qt, tb0)

            while bg or bg_slow:
                pop_bg()

    nc.compile()
    return nc


def kernel(x, mask, W_q, b_q, W_k, b_k, W_v, b_v, W_o, b_o):
    global _CACHED_NC, LAST_EXEC_NS
    bf16 = ml_dtypes.bfloat16
    f8 = ml_dtypes.float8_e4m3
    x = np.asarray(x, np.float32)
    mask = np.asarray(mask)

    xT = np.ascontiguousarray(x.reshape(BL, D).T)
    xT_hi = xT.astype(f8)
    xT_lo = (xT - xT_hi.astype(np.float32)).astype(f8)
    xT_hl = np.ascontiguousarray(np.concatenate([xT_hi, xT_lo], axis=0))
    wo_t = np.ascontiguousarray(np.asarray(W_o, np.float32).T).astype(bf16)
    bo = np.asarray(b_o, np.float32).reshape(1, D)
    pb = np.where(mask != 0, 0.0, NEG).astype(np.float32)        # [B, L]
    pad = np.ascontiguousarray(
        pb.reshape(B, NKB, KB).transpose(2, 0, 1).reshape(KB, B * NKB))
    kp = np.arange(KB)[:, None]
    qs = np.arange(KB)[None, :]
    cm = (qs >= kp).astype(np.float32).astype(bf16)   # [128,128] triangle
    cm2 = np.ascontiguousarray(np.concatenate([cm, cm], axis=1))  # [128,256]

    in_maps = []
    for c in range(NCORES):
        sl = slice(DL * c, DL * (c + 1))
        bv = np.asarray(b_v, np.float32)[sl].reshape(1, DL)
        wv_t = np.ascontiguousarray(np.asarray(W_v, np.float32)[sl].T)
        wv_hi = wv_t.astype(f8)
        wv_hi16 = (wv_hi.astype(np.float32) * 16.0).astype(f8)
        in_maps.append({
            "xT_hl": xT_hl, "wo_t": wo_t, "bo_r": bo,
            "pad_b": pad, "cmask2": cm2,
            "wq8": np.ascontiguousarray(
                np.asarray(W_q, np.float32)[sl].T).astype(f8),
            "wk8": np.ascontiguousarray(
                np.asarray(W_k, np.float32)[sl].T).astype(f8),
            "wv8": wv_hi16,
            "wv8_lo": ((wv_t - wv_hi.astype(np.float32)) * 16.0).astype(f8),
            "bq_c": np.asarray(b_q, np.float32)[sl].reshape(DL, 1),
            "bk_c": np.asarray(b_k, np.float32)[sl].reshape(DL, 1),
            "bv4_r": np.ascontiguousarray(np.tile(bv, (1, 4))),
        })

    if _CACHED_NC is None:
        _CACHED_NC = build_program()
    res = run_bass_kernel_spmd(_CACHED_NC, in_maps, list(range(NCORES)),
                               trace=TRACE)
    LAST_EXEC_NS = res.exec_time_ns
    # out_chunk rows: [256b:256b+256] = batch-b tokens [2048b+256c : +256)
    # (batch 3 split into two 128-row chunks at rows 768 and 896).
    # out_chunkT cols [64*ck : 64*(ck+1)] (ck = 4b+qt) hold the transposed
    # output for global tokens [2048b + 512qt + 64c : +64) on core c
    out = np.empty((BL, D), np.float32)
    for c in range(NCORES):
        ocT = res.results[c]["out_chunkT"]
        for ck in range(16):
            b, qt = divmod(ck, 4)
            t0 = 2048 * b + 512 * qt + 64 * c
            out[t0:t0 + 64] = ocT[:, 64 * ck:64 * (ck + 1)].T
    return np.ascontiguousarray(out.reshape(B, L, D))


# revision 72
# speedup vs baseline: 1.0041x; 1.0041x over previous
"""Causal self-attention on 8 trn2 NeuronCores.

Sharding: tensor-parallel over heads. Core c computes Q/K/V and attention
for heads {2c, 2c+1} over all batches (column-parallel W_q/W_k/W_v slices),
then per-batch AllToAlls redistribute the attention outputs token-wise so
each core runs the full output projection (contraction over all 16 heads'
features) for its 1/8 slice of each batch's tokens.

Core layout choices (per core):
 - QKV projections run in fp8 DoubleRow mode (K_eff=256/pass): x is split
   host-side into hi+lo e4m3 parts (hi+lo ~ bf16 accuracy), weights are
   single e4m3; 8 DoubleRow passes replace 8 bf16 passes at half cost.
 - Q/K are stored transposed in fp8 with a zeroed second DoubleRow slot:
   score matmuls [k,q] run DoubleRow over (real, zero) slots, halving the
   per-column cost despite the hd=64 contraction.
 - Scores psum is [128k, 2 heads, 512q]; one Exp activation covers both
   heads (bias = key-padding per partition, scale = 1/sqrt(hd)).
 - P@V runs q-major: probs block [128k,128q] is the stationary operand,
   V [128k, 64f|1] the moving one, accumulating [128q, 4sb, 65] in psum.
   The ones-column yields per-q (per-partition) softmax denominators, so
   normalization is one reciprocal + one broadcast-multiply on DVE.
 - att is stored token-major; per-batch AllToAlls move 128-token blocks,
   and the receiver transposes [tok,f]->[f,tok] with XBAR DMA transposes
   feeding the bf16 output projection. Biases ride DVE copies (no bias
   matmuls anywhere).
 - Causal masking is structural: fully-masked regions are never computed
   (restricted matmul/exp column ranges); the diagonal 128x128 triangle
   is a 0/1 multiply on GpSimd.
"""

import numpy as np
import ml_dtypes

import concourse.bass as bass
import concourse.mybir as mybir
import concourse.tile as tile
from concourse import bacc
from concourse.bass_utils import run_bass_kernel_spmd

B, L, D, H, HD = 4, 2048, 1024, 16, 64
NCORES = 8
DL = 128              # local feature dim: 2 heads * 64
BL = B * L            # 8192
SCALE = HD ** -0.5
NEG = -1e9

QT = 512              # query tile (free dim)
KB = 128              # key block (partition dim)
NQT = L // QT         # 4 q-tiles per batch
NKB = L // KB         # 16 k-blocks per batch
NTB = BL // KB        # 64 token blocks of 128

FP32 = mybir.dt.float32
BF16 = mybir.dt.bfloat16
FP8 = mybir.dt.float8e4
EXP = mybir.ActivationFunctionType.Exp
DR = mybir.MatmulPerfMode.DoubleRow

TRACE = False
LAST_EXEC_NS = None
_CACHED_NC = None
_SIM_MODE = False   # replace collectives with local DMAs; 1 device

# A2A is per q-tile: chunk ck = 4*b + qt covers that q-tile's 512 tokens;
# core c receives tokens [2048b + 512qt + 64c : +64) (64-row A2A chunks)
# and computes their full output projection, transposed ([dout, tok]).


def build_program():
    nc = bacc.Bacc("TRN2", target_bir_lowering=False, debug=False,
                   num_devices=(1 if _SIM_MODE else NCORES))
    xT_hl = nc.dram_tensor("xT_hl", [2 * D, BL], FP8, kind="ExternalInput").ap()
    wq8 = nc.dram_tensor("wq8", [D, DL], FP8, kind="ExternalInput").ap()
    wk8 = nc.dram_tensor("wk8", [D, DL], FP8, kind="ExternalInput").ap()
    wv8 = nc.dram_tensor("wv8", [D, DL], FP8, kind="ExternalInput").ap()
    wv8_lo = nc.dram_tensor("wv8_lo", [D, DL], FP8, kind="ExternalInput").ap()
    wo_t = nc.dram_tensor("wo_t", [D, D], BF16, kind="ExternalInput").ap()
    bq_c = nc.dram_tensor("bq_c", [DL, 1], FP32, kind="ExternalInput").ap()
    bk_c = nc.dram_tensor("bk_c", [DL, 1], FP32, kind="ExternalInput").ap()
    bv4_r = nc.dram_tensor("bv4_r", [1, 4 * DL], FP32, kind="ExternalInput").ap()
    bo_r = nc.dram_tensor("bo_r", [1, D], FP32, kind="ExternalInput").ap()
    pad_b = nc.dram_tensor("pad_b", [KB, B * NKB], FP32, kind="ExternalInput").ap()
    cmask2 = nc.dram_tensor("cmask2", [KB, 2 * KB], BF16, kind="ExternalInput").ap()
    out_chunkT = nc.dram_tensor("out_chunkT", [D, BL // NCORES], BF16,
                                kind="ExternalOutput").ap()

    with tile.TileContext(nc) as tc:
        with tc.tile_pool(name="persist", bufs=1) as persist, \
             tc.tile_pool(name="xpool", bufs=8) as xpool, \
             tc.tile_pool(name="probs", bufs=5) as probs, \
             tc.tile_pool(name="small", bufs=4) as small, \
             tc.tile_pool(name="opool", bufs=3) as opool, \
             tc.tile_pool(name="gpool", bufs=3) as gpool, \
             tc.tile_pool(name="ps", bufs=1, space="PSUM") as ps, \
             tc.tile_pool(name="dram", bufs=1, space="DRAM") as dram:

            # ---- persistent tiles ----
            wq_sb = persist.tile([128, 4, 2, 128], FP8)
            wk_sb = persist.tile([128, 4, 2, 128], FP8)
            wv_sb = persist.tile([128, 4, 2, 128], FP8)
            wvlo_sb = persist.tile([128, 4, 2, 128], FP8)
            wo_sb = persist.tile([128, 8, D], BF16)
            cm2_sb = persist.tile([KB, 2, KB], BF16)
            pad_sb = persist.tile([KB, B * NKB], FP32)
            bq_sb = persist.tile([DL, 1], FP32)
            bk_sb = persist.tile([DL, 1], FP32)
            bv_sb = persist.tile([128, 4, 2, 64], FP32)
            bo_sb = persist.tile([128, 8, 1], FP32)    # per-dout-partition
            QT_sb = persist.tile([128, 2, BL], FP8)    # [2h x 64hd, slot, tok]
            KT_sb = persist.tile([128, 2, BL], FP8)
            V_sb = persist.tile([128, NTB, 130], BF16)  # [key, kblk, VA|1|VB|1]
            att_sb = persist.tile([128, NTB, DL], BF16)  # token-major

            # zero DoubleRow slots / V ones-columns. Split across engines
            # and batches so the first scores matmul isn't gated ~20us:
            # batch 0's slice inline, later batches as background items.
            nc.vector.memset(QT_sb[:, 1, 0:L], 0.0)
            nc.gpsimd.memset(KT_sb[:, 1, 0:L], 0.0)
            nc.gpsimd.memset(V_sb[:, :, 64:65], 1.0)    # ones cols only —
            nc.gpsimd.memset(V_sb[:, :, 129:130], 1.0)  # rest is overwritten
            # trigger the Exp act-table load off the critical path
            warm = small.tile([1, 1], FP32, tag="warm")
            nc.vector.memset(warm, 0.0)
            warm2 = small.tile([1, 1], BF16, tag="warm2")
            nc.scalar.activation(warm2, warm, EXP)

            # ---- background PE work: QKV chunks / out-proj tiles ----
            def load_weights_qkv():
                nc.sync.dma_start(out=wq_sb, in_=wq8.rearrange(
                    "(t s p) m -> p t s m", p=128, s=2))
                nc.sync.dma_start(out=wk_sb, in_=wk8.rearrange(
                    "(t s p) m -> p t s m", p=128, s=2))
                nc.sync.dma_start(out=wv_sb, in_=wv8.rearrange(
                    "(t s p) m -> p t s m", p=128, s=2))
                nc.sync.dma_start(out=wvlo_sb, in_=wv8_lo.rearrange(
                    "(t s p) m -> p t s m", p=128, s=2))

            def load_consts():
                # small consts ride the (otherwise idle) Act HWDGE queue so
                # they don't serialize behind the big x/weight loads on SP
                nc.scalar.dma_start(out=bq_sb, in_=bq_c)
                nc.scalar.dma_start(out=bk_sb, in_=bk_c)
                nc.scalar.dma_start(out=pad_sb, in_=pad_b)
                nc.scalar.dma_start(out=cm2_sb, in_=cmask2.rearrange(
                    "p (h q) -> p h q", h=2))
                nc.scalar.dma_start(out=bv_sb, in_=bv4_r.rearrange(
                    "o (v h f) -> o v h f", v=4, h=2).to_broadcast(
                        [128, 4, 2, 64]))

            def load_wo():
                nc.sync.dma_start(out=wo_sb, in_=wo_t.rearrange(
                    "(t p) m -> p t m", p=128))

            def load_bo():
                nc.sync.dma_start(out=bo_sb, in_=bo_r.rearrange(
                    "o (db p) -> p db o", p=128))

            def emit_xt_dma(lc, split=False):
                xt = xpool.tile([128, 2, 4, 2, QT], FP8, tag="xt",
                                name=f"xt{lc}")
                cols = slice(QT * lc, QT * (lc + 1))
                src = xT_hl[:, cols].rearrange(
                    "(u t s p) l -> p u t s l", p=128, s=2, t=4)
                if split:   # hi first so the first matmuls start sooner
                    nc.sync.dma_start(out=xt[:, 0], in_=src[:, 0])
                    nc.sync.dma_start(out=xt[:, 1], in_=src[:, 1])
                else:
                    nc.sync.dma_start(out=xt, in_=src)
                return xt[:, 0], xt[:, 1]

            xts = {}

            def emit_chunk_qk(lc, which):
                xh, xl = xts[lc]
                w_sb, b_sb, dst = ((wq_sb, bq_sb, QT_sb) if which == "q"
                                   else (wk_sb, bk_sb, KT_sb))
                p = ps.tile([128, QT], FP32, tag="mm", bufs=2, name=f"p{which}{lc}")
                for pi, xt in enumerate((xh, xl)):
                    for t in range(4):
                        nc.tensor.matmul(p, lhsT=w_sb[:, t], rhs=xt[:, t],
                                         perf_mode=DR,
                                         start=(pi == 0 and t == 0),
                                         stop=(pi == 1 and t == 3))
                nc.vector.tensor_scalar_add(
                    dst[:, 0, QT * lc:QT * (lc + 1)], p, b_sb)

            def emit_chunk_v(lc):
                # psum accumulates 16*V: (xh+xl) @ (16*wv_hi) + xh @ wv_lo16
                # (the W residual is pre-scaled x16 host-side so it lands in
                # e4m3's range); one DVE op then does psum/16 + bias.
                xh, xl = xts[lc]
                p = ps.tile([128, 4, 128], FP32, tag="mm", bufs=2, name=f"pv{lc}")
                # one accumulation group for the whole 2KB psum zero-region:
                # start exactly once, stop exactly once
                for pi, xt, w_sb in ((0, xh, wv_sb), (1, xl, wv_sb),
                                     (2, xh, wvlo_sb)):
                    for t in range(4):
                        for vs in range(4):
                            nc.tensor.matmul(
                                p[:, vs, :],
                                lhsT=xt[:, t, :, KB * vs:KB * (vs + 1)],
                                rhs=w_sb[:, t], perf_mode=DR,
                                start=(pi == 0 and t == 0 and vs == 0),
                                stop=(pi == 2 and t == 3 and vs == 3))
                kt0 = 4 * lc
                nc.vector.scalar_tensor_tensor(
                    V_sb[:, kt0:kt0 + 4, :].rearrange(
                        "p v (h fo) -> p v h fo", h=2)[:, :, :, 0:64],
                    p.rearrange("p v (h f) -> p v h f", h=2),
                    1.0 / 16.0, bv_sb,
                    mybir.AluOpType.mult, mybir.AluOpType.add)

            gathT = {}

            def emit_op(ck):
                """Out-proj for chunk ck (64 tokens): out[dout, tok] =
                sum_f w_o[f, dout] att[f, tok] — wo stationary, 64-token
                gathT moving, transposed output."""
                g = gathT[ck]    # [128 f, 8 src, 64 tok]
                po = ps.tile([128, 8, 64], FP32, tag="mm", bufs=2,
                             name=f"po{ck}")
                for db in range(8):
                    for s in range(8):
                        nc.tensor.matmul(
                            po[:, db, :],
                            lhsT=wo_sb[:, s, KB * db:KB * (db + 1)],
                            rhs=g[:, s, :],
                            start=(db == 0 and s == 0),
                            stop=(db == 7 and s == 7))
                ot = opool.tile([128, 8, 64], BF16, tag="ot",
                                name=f"ot{ck}")
                nc.vector.tensor_add(ot, po,
                                     bo_sb.to_broadcast([128, 8, 64]))
                nc.sync.dma_start(
                    out=out_chunkT[:, 64 * ck:64 * (ck + 1)].rearrange(
                        "(db p) t -> p db t", p=128),
                    in_=ot)

            bg = []
            bg_slow = []         # (ready_at_pop, fn): out-proj work
            pop_count = [0]

            def mk(fn, *a):
                return lambda: fn(*a)

            def pop_bg(n=1):
                for _ in range(n):
                    pop_count[0] += 1
                    if bg:
                        bg.pop(0)()
                    elif bg_slow and bg_slow[0][0] <= pop_count[0]:
                        bg_slow.pop(0)[1]()

            # seed: QKV for batch 0 (chunks 0-3), prefetching 2 ahead.
            # xt(0) first so the first matmul isn't behind all const loads.
            xts[0] = emit_xt_dma(0, split=True)
            load_weights_qkv()
            xts[1] = emit_xt_dma(1)
            load_consts()
            for lc in range(4):
                if lc + 2 < 4:
                    xts[lc + 2] = emit_xt_dma(lc + 2)
                emit_chunk_qk(lc, "q")
                emit_chunk_qk(lc, "k")
                emit_chunk_v(lc)
            bg.append(load_wo)
            bg.append(load_bo)

            def push_batch_chunks(b):
                def dma_item(l):
                    return mk(lambda l2: xts.__setitem__(l2, emit_xt_dma(l2)), l)
                l0 = 4 * b
                cols = slice(L * b, L * (b + 1))
                bg.append(mk(nc.gpsimd.memset, QT_sb[:, 1, cols], 0.0))
                bg.append(mk(nc.gpsimd.memset, KT_sb[:, 1, cols], 0.0))
                for lc in range(l0, l0 + 4):
                    bg.append(dma_item(lc))
                for lc in range(l0, l0 + 4):
                    bg.append(mk(emit_chunk_qk, lc, "q"))
                    bg.append(mk(emit_chunk_qk, lc, "k"))
                    bg.append(mk(emit_chunk_v, lc))

            def emit_a2a(ck, tb0):
                """A2A + transpose for q-tile chunk ck (tbs tb0..tb0+4):
                64-row chunks; core c receives tokens [64c:64c+64) of the
                tile from every source core."""
                a_in = dram.tile([NCORES * 64, KB], BF16, tag=f"ai{ck}",
                                 name=f"a2a_in{ck}")
                a_out = dram.tile([NCORES * 64, KB], BF16, tag=f"ao{ck}",
                                  name=f"a2a_out{ck}")
                nc.sync.dma_start(
                    out=a_in.rearrange("(tb p) f -> p tb f", p=128),
                    in_=att_sb[:, tb0:tb0 + 4, :])
                if _SIM_MODE:
                    nc.sync.dma_start(out=a_out, in_=a_in)
                else:
                    nc.gpsimd.collective_compute(
                        "AllToAll", mybir.AluOpType.bypass,
                        replica_groups=[list(range(NCORES))],
                        ins=[a_in.opt()], outs=[a_out.opt()])
                # XBAR transpose [512 (s,tok), 128 f] -> [128 f, 8 s, 64 tok]
                # (HW semantics: out[p, e, l] = in[e*L + l, p], L=64; note
                # the interpreter disagrees — hardware-verified via probe)
                g = gpool.tile([128, 8, 64], BF16, tag="gt",
                               name=f"gathT{ck}")
                gathT[ck] = g
                nc.sync.dma_start_transpose(out=g, in_=a_out)
                bg_slow.append((pop_count[0] + 24, mk(emit_op, ck)))

            # ---- attention ----
            for b in range(B):
                if b + 1 < B:
                    push_batch_chunks(b + 1)
                for qt in range(NQT):
                    q0 = L * b + QT * qt
                    tb0 = 16 * b + 4 * qt
                    nkb = 4 * (qt + 1)
                    pva = ps.tile([128, 4, 65], FP32, tag="pva", name=f"pva{b}{qt}")
                    pvb = ps.tile([128, 4, 65], FP32, tag="pvb", name=f"pvb{b}{qt}")
                    pvs = (pva, pvb)
                    for j in range(nkb):
                        k0 = L * b + KB * j
                        kt = NKB * b + j
                        o = j - 4 * qt
                        c0 = KB * o if o >= 0 else 0
                        sc = ps.tile([128, 2, QT], FP32, tag="sc", bufs=2,
                                     name=f"sc{b}{qt}{j}")
                        # scores feed the Act-bound exp pipeline: bias the
                        # scheduler to place them ahead of background work
                        with tc.high_priority(300):
                            for h, p0 in ((0, 0), (1, 64)):
                                nc.tensor.matmul(
                                    sc[:, h, c0:], perf_mode=DR,
                                    lhsT=KT_sb[p0:p0 + 64, :, k0:k0 + KB],
                                    rhs=QT_sb[p0:p0 + 64, :, q0 + c0:q0 + QT],
                                    start=True, stop=True)
                        pr = probs.tile([128, 2, QT], BF16, tag="pr",
                                        name=f"pr{b}{qt}{j}")
                        nc.scalar.activation(pr[:, :, c0:], sc[:, :, c0:],
                                             EXP, bias=pad_sb[:, kt:kt + 1],
                                             scale=SCALE)
                        if o >= 0:
                            nc.gpsimd.tensor_mul(pr[:, :, c0:c0 + KB],
                                                 pr[:, :, c0:c0 + KB], cm2_sb)
                        for h in range(2):
                            for sb in range(max(o, 0), 4):
                                nc.tensor.matmul(
                                    pvs[h][:, sb, :],
                                    lhsT=pr[:, h, KB * sb:KB * (sb + 1)],
                                    rhs=V_sb[:, kt, 65 * h:65 * (h + 1)],
                                    start=(j == 0 and sb == 0),
                                    stop=(j == nkb - 1 and sb == 3))
                        pop_bg()
                    for h in range(2):
                        rec = small.tile([128, 4, 1], FP32, tag="rec",
                                         name=f"rec{b}{qt}{h}")
                        nc.vector.reciprocal(rec, pvs[h][:, :, 64:65])
                        nc.vector.tensor_mul(
                            att_sb[:, tb0:tb0 + 4, 64 * h:64 * (h + 1)],
                            pvs[h][:, :, 0:64],
                            rec.to_broadcast([128, 4, 64]))
                    emit_a2a(4 * b + qt, tb0)

            while bg or bg_slow:
                pop_bg()

    nc.compile()
    return nc


def kernel(x, mask, W_q, b_q, W_k, b_k, W_v, b_v, W_o, b_o):
    global _CACHED_NC, LAST_EXEC_NS
    bf16 = ml_dtypes.bfloat16
    f8 = ml_dtypes.float8_e4m3
    x = np.asarray(x, np.float32)
    mask = np.asarray(mask)

    xT = np.ascontiguousarray(x.reshape(BL, D).T)
    xT_hi = xT.astype(f8)
    xT_lo = (xT - xT_hi.astype(np.float32)).astype(f8)
    xT_hl = np.ascontiguousarray(np.concatenate([xT_hi, xT_lo], axis=0))
    wo_t = np.ascontiguousarray(np.asarray(W_o, np.float32).T).astype(bf16)
    bo = np.asarray(b_o, np.float32).reshape(1, D)
    pb = np.where(mask != 0, 0.0, NEG).astype(np.float32)        # [B, L]
    pad = np.ascontiguousarray(
        pb.reshape(B, NKB, KB).transpose(2, 0, 1).reshape(KB, B * NKB))
    kp = np.arange(KB)[:, None]
    qs = np.arange(KB)[None, :]
    cm = (qs >= kp).astype(np.float32).astype(bf16)   # [128,128] triangle
    cm2 = np.ascontiguousarray(np.concatenate([cm, cm], axis=1))  # [128,256]

    in_maps = []
    for c in range(NCORES):
        sl = slice(DL * c, DL * (c + 1))
        bv = np.asarray(b_v, np.float32)[sl].reshape(1, DL)
        wv_t = np.ascontiguousarray(np.asarray(W_v, np.float32)[sl].T)
        wv_hi = wv_t.astype(f8)
        wv_hi16 = (wv_hi.astype(np.float32) * 16.0).astype(f8)
        in_maps.append({
            "xT_hl": xT_hl, "wo_t": wo_t, "bo_r": bo,
            "pad_b": pad, "cmask2": cm2,
            "wq8": np.ascontiguousarray(
                np.asarray(W_q, np.float32)[sl].T).astype(f8),
            "wk8": np.ascontiguousarray(
                np.asarray(W_k, np.float32)[sl].T).astype(f8),
            "wv8": wv_hi16,
            "wv8_lo": ((wv_t - wv_hi.astype(np.float32)) * 16.0).astype(f8),
            "bq_c": np.asarray(b_q, np.float32)[sl].reshape(DL, 1),
            "bk_c": np.asarray(b_k, np.float32)[sl].reshape(DL, 1),
            "bv4_r": np.ascontiguousarray(np.tile(bv, (1, 4))),
        })

    if _CACHED_NC is None:
        _CACHED_NC = build_program()
    res = run_bass_kernel_spmd(_CACHED_NC, in_maps, list(range(NCORES)),
                               trace=TRACE)
    LAST_EXEC_NS = res.exec_time_ns
    # out_chunk rows: [256b:256b+256] = batch-b tokens [2048b+256c : +256)
    # (batch 3 split into two 128-row chunks at rows 768 and 896).
    # out_chunkT cols [64*ck : 64*(ck+1)] (ck = 4b+qt) hold the transposed
    # output for global tokens [2048b + 512qt + 64c : +64) on core c
    out = np.empty((BL, D), np.float32)
    for c in range(NCORES):
        ocT = res.results[c]["out_chunkT"]
        for ck in range(16):
            b, qt = divmod(ck, 4)
            t0 = 2048 * b + 512 * qt + 64 * c
            out[t0:t0 + 64] = ocT[:, 64 * ck:64 * (ck + 1)].T
    return np.ascontiguousarray(out.reshape(B, L, D))
